# revision 24
# baseline (speedup 1.0000x reference)
"""Per-domain batch normalization (BaseDomainBatchNorm) on 8 Trainium2 NeuronCores.

Math (reference):
    cnt[j]   = #{n : d[n] == j}            (clamped to >= 1)
    mean[j]  = sum_{d[n]==j} X[n] / cnt[j]
    var[j]   = sum_{d[n]==j} X[n]^2 / cnt[j] - mean[j]^2
    inv[j]   = rsqrt(var[j] + 1e-5)
    Y[n]     = (X[n] - mean[d[n]]) * inv[d[n]] * gamma[d[n]] + beta[d[n]]
             = X[n] * A[d[n]] + B[d[n]],  A = inv*gamma, B = beta - mean*A

Sharding: rows split 8192 per core; per-domain partial stats (sum/sumsq)
AllReduce'd across the 8 cores; each core normalizes its own rows.

v4 design
---------
Stats path (row-major, original order): host uploads X and X^2 in fp8 e4m3
plus the row one-hot of d; partial sums are 64 fp8 DoubleRow matmuls
(2 chunks = 2 K-tiles per pass).  Counts are exact on the host; 1/cnt
ships as a constant.  The AllReduce payload is [16, 2C+1] f32 (65600 B —
kept above 64 KiB so the runtime picks the faster RDH algorithm).

Normalize path (transposed, domain-sorted): host sorts rows by domain and
uploads X^T in fp16 (channels on partitions) with each domain's rows
padded to a fixed per-core allocation; every (channel-block, domain)
rectangle is a compile-time slice whose A/B are per-partition [128,1]
scalars, so the normalize is ONE fused mul-add per rectangle (DVE
tensor_scalar ~0.8ns/elem, shared 2:1 with the scalar engine's Identity
activation).  Y^T returns in fp16 and is unsorted/upcast on the host.

The finalize runs in the transposed [128 x 64] layout: the reduced stats
are PE-transposed against a [16,16] identity right after the AllReduce,
gamma/beta/1-over-cnt arrive pre-transposed from the host, and the
reciprocal runs on 128 partitions (8x shorter).

DMA discipline (one queue per DMA instruction, ~23 GB/s each): stats
tiles are 16+16 x 256 KB interleaved; X^T rides the same sync-engine
queues BEHIND them (FIFO) as 32 pair-tiles; Y writes are 32 pair-tiles.
The collective input/output DMAs trigger from the scalar engine so they
never queue behind bulk traffic.  Program is compiled per domain-
allocation tuple (derived from d) and cached.
"""

import numpy as np

N = 65536
C = 512
D = 16
NCORES = 8
SHARD = N // NCORES          # 8192 rows per core
P = 128                      # partitions
CHUNKS = SHARD // P          # 64 chunks of 128 rows
PAIRS = CHUNKS // 2          # 32 DoubleRow K-tile pairs
LOAD_CH = 4                  # chunks per stats DMA tile
CB = C // P                  # 4 channel blocks
EPS = 1e-5

_CACHE = {}


def _build_program(alloc):
    import concourse.bacc as bacc
    import concourse.tile as tile
    from concourse import mybir

    f32 = mybir.dt.float32
    f16 = mybir.dt.float16
    fp8 = mybir.dt.float8e4
    i32 = mybir.dt.int32
    Alu = mybir.AluOpType
    Act = mybir.ActivationFunctionType

    padrows = sum(alloc)
    offs = np.concatenate([[0], np.cumsum(alloc)]).astype(int)

    nc = bacc.Bacc("TRN2", target_bir_lowering=False, debug=False,
                   num_devices=NCORES)

    Xs_d = nc.dram_tensor("Xs", [SHARD, C], fp8, kind="ExternalInput")
    X2_d = nc.dram_tensor("X2", [SHARD, C], fp8, kind="ExternalInput")
    OH_d = nc.dram_tensor("OH", [P, CHUNKS * D], fp8, kind="ExternalInput")
    XT_d = nc.dram_tensor("XT", [C, padrows], f16, kind="ExternalInput")
    ri_d = nc.dram_tensor("rinvT", [P, CB * D], f32, kind="ExternalInput")
    g_d = nc.dram_tensor("gammaT", [P, CB * D], f32, kind="ExternalInput")
    b_d = nc.dram_tensor("betaT", [P, CB * D], f32, kind="ExternalInput")
    YT_d = nc.dram_tensor("YT", [C, padrows], f16, kind="ExternalOutput")

    cc_in = nc.dram_tensor("cc_in", [D, 2 * C + 1], f32)
    cc_out = nc.dram_tensor("cc_out", [D, 2 * C + 1], f32,
                            addr_space="Shared")

    # stats layout: partition p owns rows [p*64, (p+1)*64); chunk i = rows
    # {p*64 + i}; a LOAD_CH tile is 4 consecutive rows -> 2KB contiguous
    Xv = Xs_d.ap().rearrange("(p n) c -> p n c", p=P)   # [128, 64, 512]
    X2v = X2_d.ap().rearrange("(p n) c -> p n c", p=P)

    with tile.TileContext(nc) as tc:
        with (
            tc.tile_pool(name="const", bufs=1) as cpool,
            tc.tile_pool(name="xt", bufs=CB * D // 2) as xtpool,
            tc.tile_pool(name="small", bufs=1) as spool,
            tc.tile_pool(name="scr", bufs=2) as scrpool,
        ):
            # ---- small loads + constants ----
            oh = cpool.tile([P, CHUNKS, D], fp8)
            nc.sync.dma_start(oh[:], OH_d.ap().rearrange(
                "p (n d) -> p n d", d=D))
            rinvT = spool.tile([P, CB * D], f32, tag="rinvT")
            nc.sync.dma_start(rinvT[:], ri_d[:])
            gamT = spool.tile([P, CB * D], f32, tag="gamT")
            nc.sync.dma_start(gamT[:], g_d[:])
            betT = spool.tile([P, CB * D], f32, tag="betT")
            nc.sync.dma_start(betT[:], b_d[:])
            # identity [16,16] f32 for the PE transposes
            iota_r = cpool.tile([D, D], i32)
            nc.gpsimd.iota(iota_r[:], pattern=[[1, D]], base=0,
                           channel_multiplier=0)
            iota_c = cpool.tile([D, 1], i32)
            nc.gpsimd.iota(iota_c[:], pattern=[[0, 1]], base=0,
                           channel_multiplier=1)
            iota_cf = cpool.tile([D, 1], f32)
            nc.vector.tensor_copy(iota_cf[:], iota_c[:])
            ident = cpool.tile([D, D], f32)
            nc.vector.tensor_scalar(ident[:], iota_r[:], iota_cf[:], None,
                                    Alu.is_equal)
            epsb = spool.tile([P, 1], f32, tag="epsb")
            nc.vector.memset(epsb[:], EPS)

            # ---- stats inputs: Xs/X2 4-chunk tiles, interleaved so both
            # streams of a pair arrive together; the x pool is scoped so
            # its 64KB/partition is reused by the phase-2 y pool ----
            with tc.tile_pool(name="x", bufs=12) as xpool:
                xs, x2 = [], []
                for s in range(CHUNKS // LOAD_CH):
                    t1 = xpool.tile([P, LOAD_CH * C], fp8)
                    xs.append(t1)
                    nc.sync.dma_start(
                        t1[:].rearrange("p (n c) -> p n c", c=C),
                        Xv[:, LOAD_CH * s:LOAD_CH * (s + 1), :])
                    t2 = xpool.tile([P, LOAD_CH * C], fp8)
                    x2.append(t2)
                    # scalar-engine trigger: issues in parallel with sync's
                    nc.scalar.dma_start(
                        t2[:].rearrange("p (n c) -> p n c", c=C),
                        X2v[:, LOAD_CH * s:LOAD_CH * (s + 1), :])

                # X^T pair-tiles ride the same sync queues AFTER Xs/X2
                xtt = {}
                for cb in range(CB):
                    for jp in range(D // 2):
                        j0 = 2 * jp
                        w = int(alloc[j0] + alloc[j0 + 1])
                        t = xtpool.tile([P, w], f16)
                        xtt[(cb, jp)] = t
                        nc.sync.dma_start(
                            t[:], XT_d.ap()[cb * P:(cb + 1) * P,
                                            offs[j0]:offs[j0 + 2]])

                def pair(lst, k):  # [128, 2, C] fp8 for chunks 2k, 2k+1
                    t = lst[(2 * k) // LOAD_CH]
                    o = ((2 * k) % LOAD_CH) * C
                    return t[:, o:o + 2 * C].rearrange("p (t c) -> p t c",
                                                       c=C)

                # phase 1: per-core partial sums via fp8 DoubleRow
                with tc.tile_pool(name="ps1", bufs=1, space="PSUM") as ps1:
                    psum_s = ps1.tile([D, C], f32)
                    psum_q = ps1.tile([D, C], f32)
                    for k in range(PAIRS):
                        st, sp = (k == 0), (k == PAIRS - 1)
                        lhs = oh[:, 2 * k:2 * k + 2, :]
                        nc.tensor.matmul(
                            psum_s[:], lhs, pair(xs, k), start=st, stop=sp,
                            perf_mode=mybir.MatmulPerfMode.DoubleRow)
                        nc.tensor.matmul(
                            psum_q[:], lhs, pair(x2, k), start=st, stop=sp,
                            perf_mode=mybir.MatmulPerfMode.DoubleRow)

                    # stats out: PSUM -> SBUF copies, then two parallel
                    # half-DMAs to cc_in on different trigger engines
                    stats = spool.tile([D, 2 * C + 1], f32, tag="stats")
                    nc.vector.memset(stats[:, 2 * C:2 * C + 1], 0.0)
                    nc.vector.tensor_copy(stats[:, 0:C], psum_s[:])
                    nc.vector.tensor_copy(stats[:, C:2 * C], psum_q[:])
                    # gpsimd queues are empty in phase 1 -> no FIFO backlog
                    nc.gpsimd.dma_start(cc_in.ap()[:, 0:C], stats[:, 0:C])
                    nc.scalar.dma_start(cc_in.ap()[:, C:2 * C + 1],
                                        stats[:, C:2 * C + 1])

                    # keep the PE pstate up across the all-reduce stall so
                    # the post-AR transposes run at speed
                    warm = ps1.tile([D, C], f32)
                    for _ in range(16):
                        nc.tensor.matmul(warm[:], oh[:, 0:2, :],
                                         pair(xs, 0), start=True, stop=True,
                                         perf_mode=mybir.MatmulPerfMode.DoubleRow,
                                         skip_group_check=True)

            # ---- all-reduce partial sums ----
            nc.gpsimd.collective_compute(
                "AllReduce", Alu.add,
                replica_groups=[list(range(NCORES))],
                ins=[cc_in[:]], outs=[cc_out[:]])
            # read the result back in two parallel halves (gpsimd/scalar:
            # their queues are clear of bulk traffic by now)
            red = spool.tile([D, 2 * C], f32, tag="red")
            nc.scalar.dma_start(red[:, 0:C], cc_out.ap()[:, 0:C])
            nc.gpsimd.dma_start(red[:, C:2 * C], cc_out.ap()[:, C:2 * C])

            # ---- transpose reduced stats: redT[p, b*16+j] = red[j, b*128+p]
            redT = spool.tile([P, 2 * CB * D], f32, tag="redT")
            with tc.tile_pool(name="ps2", bufs=1, space="PSUM") as ps2:
                pt = ps2.tile([P, 2 * CB * D], f32)
                for b in range(2 * CB):
                    nc.tensor.matmul(pt[:, b * D:(b + 1) * D],
                                     red[:, b * P:(b + 1) * P], ident[:],
                                     start=True, stop=True)
                nc.vector.tensor_copy(redT[:], pt[:])

            # ---- finalize in [128, 64]: A = inv*gamma, B = beta - mean*A
            F = CB * D
            meanT = spool.tile([P, F], f32, tag="meanT")
            nc.vector.tensor_mul(meanT[:], redT[:, 0:F], rinvT[:])
            varT = spool.tile([P, F], f32, tag="varT")
            nc.vector.tensor_mul(varT[:], redT[:, F:2 * F], rinvT[:])
            negm2 = scrpool.tile([P, F], f32, tag="scr")
            nc.vector.scalar_tensor_tensor(negm2[:], meanT[:], -1.0, meanT[:],
                                           Alu.mult, Alu.mult)
            nc.vector.tensor_add(varT[:], varT[:], negm2[:])
            sdT = scrpool.tile([P, F], f32, tag="scr")
            nc.scalar.activation(sdT[:], varT[:], Act.Sqrt, bias=epsb[:])
            invT = spool.tile([P, F], f32, tag="invT")
            nc.vector.reciprocal(invT[:], sdT[:])
            AT = spool.tile([P, F], f32, tag="AT")
            nc.vector.tensor_mul(AT[:], invT[:], gamT[:])
            BT = spool.tile([P, F], f32, tag="BT")
            nc.vector.scalar_tensor_tensor(BT[:], meanT[:], -1.0, AT[:],
                                           Alu.mult, Alu.mult)   # -mean*A
            nc.vector.tensor_add(BT[:], betT[:], BT[:])

            def a_col(cb, j):
                return AT[:, cb * D + j:cb * D + j + 1]

            def b_col(cb, j):
                return BT[:, cb * D + j:cb * D + j + 1]

            # ---- phase 2: y^T = x^T * A[d,c] + B[d,c] per rectangle ----
            # greedy DVE/ACT balance (ns/col measured on hw; DVE starts
            # loaded with the finalize chain)
            with tc.tile_pool(name="y", bufs=16) as ypool:
                dve_t, act_t = 2500.0, 0.0
                for cb in range(CB):
                    for jp in range(D // 2):
                        j0 = 2 * jp
                        w0 = int(alloc[j0])
                        w = int(alloc[j0] + alloc[j0 + 1])
                        xt2 = xtt[(cb, jp)]
                        yt = ypool.tile([P, w], f16)
                        for h, j in ((0, j0), (1, j0 + 1)):
                            sl = slice(0, w0) if h == 0 else slice(w0, w)
                            wj = int(alloc[j])
                            cd, ca = wj * 0.807, wj * 1.72
                            if dve_t + cd <= act_t + ca:
                                dve_t += cd
                                nc.vector.tensor_scalar(yt[:, sl], xt2[:, sl],
                                                        a_col(cb, j),
                                                        b_col(cb, j),
                                                        Alu.mult, Alu.add)
                            else:
                                act_t += ca
                                nc.scalar.activation(yt[:, sl], xt2[:, sl],
                                                     Act.Identity,
                                                     bias=b_col(cb, j),
                                                     scale=a_col(cb, j))
                        eng = nc.sync if jp % 2 == 0 else nc.gpsimd
                        eng.dma_start(
                            YT_d.ap()[cb * P:(cb + 1) * P,
                                      offs[j0]:offs[j0 + 2]],
                            yt[:])

    nc.compile()
    return nc


def _get_program(alloc):
    key = tuple(int(a) for a in alloc)
    if key not in _CACHE:
        _CACHE[key] = _build_program(alloc)
    return _CACHE[key]


def _plan(d):
    """Per-core, per-domain row assignment (SPMD-uniform allocation)."""
    cnt = np.bincount(d, minlength=D).astype(np.int64)
    # per-core allocation for domain j, rounded up to 32 rows, min 32
    alloc = np.maximum(32, ((cnt + NCORES - 1) // NCORES + 31) // 32 * 32)
    order = np.argsort(d, kind="stable")
    splits = np.cumsum(cnt)[:-1]
    by_dom = np.split(order, splits)          # global row ids per domain
    padrows = int(alloc.sum())
    perm = np.empty((NCORES, padrows), dtype=np.int64)
    valid = np.zeros((NCORES, padrows), dtype=bool)
    offs = np.concatenate([[0], np.cumsum(alloc)]).astype(int)
    for j in range(D):
        rows = by_dom[j]
        cuts = np.linspace(0, len(rows), NCORES + 1).astype(np.int64)
        for c in range(NCORES):
            part = rows[cuts[c]:cuts[c + 1]]
            n = len(part)
            o = offs[j]
            if n:
                perm[c, o:o + n] = part
                valid[c, o:o + n] = True
                perm[c, o + n:offs[j + 1]] = part[0]  # pad = repeat
            else:
                perm[c, o:offs[j + 1]] = 0            # inert; masked out
    return alloc, perm, valid, cnt


def _prepare(X, d, gamma, beta):
    """Build (nc, in_maps, plan) for the given full inputs."""
    import ml_dtypes

    X = np.ascontiguousarray(np.asarray(X), dtype=np.float32)
    d = np.ascontiguousarray(np.asarray(d), dtype=np.int32)
    gamma = np.ascontiguousarray(np.asarray(gamma), dtype=np.float32)
    beta = np.ascontiguousarray(np.asarray(beta), dtype=np.float32)

    alloc, perm, valid, cnt = _plan(d)
    nc = _get_program(alloc)

    X8 = X.astype(ml_dtypes.float8_e4m3)
    X28 = (X * X).astype(ml_dtypes.float8_e4m3)
    Xh = X.astype(np.float16)
    rinv = (1.0 / np.maximum(cnt, 1)).astype(np.float32)

    # transposed per-partition constants: t[p, cb*16+j] over channels
    # c = cb*128+p
    def tconst(M):  # M [D, C] -> [P, CB*D]
        out = np.empty((P, CB * D), dtype=np.float32)
        for cb in range(CB):
            out[:, cb * D:(cb + 1) * D] = M[:, cb * P:(cb + 1) * P].T
        return np.ascontiguousarray(out)

    rinvT = np.ascontiguousarray(
        np.tile(rinv[None, :], (P, CB)).astype(np.float32))
    gamT = tconst(gamma)
    betT = tconst(beta)

    in_maps = []
    for c in range(NCORES):
        ds = d[c * SHARD:(c + 1) * SHARD]
        dv = ds.reshape(P, CHUNKS)                      # row p*64+i
        ohc = (dv[:, :, None] == np.arange(D)[None, None, :])
        ohc = np.ascontiguousarray(
            ohc.reshape(P, CHUNKS * D).astype(ml_dtypes.float8_e4m3))
        xtc = np.ascontiguousarray(Xh[perm[c]].T)       # [C, padrows] f16
        in_maps.append({
            "Xs": X8[c * SHARD:(c + 1) * SHARD],
            "X2": X28[c * SHARD:(c + 1) * SHARD],
            "OH": ohc,
            "XT": xtc,
            "rinvT": rinvT,
            "gammaT": gamT,
            "betaT": betT,
        })
    return nc, in_maps, (perm, valid)


def _unpack(res, plan):
    perm, valid = plan
    Y = np.empty((N, C), dtype=np.float32)
    for c in range(NCORES):
        yt = np.asarray(res.results[c]["YT"]).astype(np.float32)  # [C, pad]
        m = valid[c]
        Y[perm[c][m]] = yt.T[m]
    return Y


def kernel(X, d, parameter_t, fm_mean, gamma, beta):
    from concourse.bass_utils import run_bass_kernel_spmd

    nc, in_maps, plan = _prepare(X, d, gamma, beta)
    res = run_bass_kernel_spmd(nc, in_maps, core_ids=list(range(NCORES)))
    return _unpack(res, plan)


# revision 25
# speedup vs baseline: 1.4905x; 1.4905x over previous
"""Per-domain batch normalization (BaseDomainBatchNorm) on 8 Trainium2 NeuronCores.

Math (reference):
    cnt[j]   = #{n : d[n] == j}            (clamped to >= 1)
    mean[j]  = sum_{d[n]==j} X[n] / cnt[j]
    var[j]   = sum_{d[n]==j} X[n]^2 / cnt[j] - mean[j]^2
    inv[j]   = rsqrt(var[j] + 1e-5)
    Y[n]     = (X[n] - mean[d[n]]) * inv[d[n]] * gamma[d[n]] + beta[d[n]]
             = X[n] * A[d[n]] + B[d[n]],  A = inv*gamma, B = beta - mean*A

Sharding: rows split 8192 per core; per-domain partial stats (sum/sumsq)
AllReduce'd across the 8 cores; each core normalizes its own rows.

v4 design
---------
Stats path (row-major, original order): host uploads X and X^2 in fp8 e4m3
plus the row one-hot of d; partial sums are 64 fp8 DoubleRow matmuls
(2 chunks = 2 K-tiles per pass).  Counts are exact on the host; 1/cnt
ships as a constant.  The AllReduce payload is [16, 2C+1] f32 (65600 B —
kept above 64 KiB so the runtime picks the faster RDH algorithm).

Normalize path (transposed, domain-sorted): host sorts rows by domain and
uploads X^T in fp16 (channels on partitions) with each domain's rows
padded to a fixed per-core allocation; every (channel-block, domain)
rectangle is a compile-time slice whose A/B are per-partition [128,1]
scalars, so the normalize is ONE fused mul-add per rectangle (DVE
tensor_scalar ~0.8ns/elem, shared 2:1 with the scalar engine's Identity
activation).  Y^T returns in fp16 and is unsorted/upcast on the host.

The finalize runs in the transposed [128 x 64] layout: the reduced stats
are PE-transposed against a [16,16] identity right after the AllReduce,
gamma/beta/1-over-cnt arrive pre-transposed from the host, and the
reciprocal runs on 128 partitions (8x shorter).

DMA discipline (one queue per DMA instruction, ~23 GB/s each): stats
tiles are 16+16 x 256 KB interleaved; X^T rides the same sync-engine
queues BEHIND them (FIFO) as 32 pair-tiles; Y writes are 32 pair-tiles.
The collective input/output DMAs trigger from the scalar engine so they
never queue behind bulk traffic.  Program is compiled per domain-
allocation tuple (derived from d) and cached.
"""

import numpy as np

N = 65536
C = 512
D = 16
NCORES = 8
SHARD = N // NCORES          # 8192 rows per core
P = 128                      # partitions
CHUNKS = SHARD // P          # 64 chunks of 128 rows
PAIRS = CHUNKS // 2          # 32 DoubleRow K-tile pairs
LOAD_CH = 4                  # chunks per stats DMA tile
CB = C // P                  # 4 channel blocks
EPS = 1e-5

_CACHE = {}


def _build_program(alloc):
    import concourse.bacc as bacc
    import concourse.tile as tile
    from concourse import mybir

    f32 = mybir.dt.float32
    f16 = mybir.dt.float16
    fp8 = mybir.dt.float8e4
    i32 = mybir.dt.int32
    Alu = mybir.AluOpType
    Act = mybir.ActivationFunctionType

    padrows = sum(alloc)
    offs = np.concatenate([[0], np.cumsum(alloc)]).astype(int)

    nc = bacc.Bacc("TRN2", target_bir_lowering=False, debug=False,
                   num_devices=NCORES)

    Xs_d = nc.dram_tensor("Xs", [SHARD, C], fp8, kind="ExternalInput")
    X2_d = nc.dram_tensor("X2", [SHARD, C], fp8, kind="ExternalInput")
    OH_d = nc.dram_tensor("OH", [P, CHUNKS * D], fp8, kind="ExternalInput")
    XT_d = nc.dram_tensor("XT", [C, padrows], f16, kind="ExternalInput")
    ri_d = nc.dram_tensor("rinvT", [P, CB * D], f32, kind="ExternalInput")
    g_d = nc.dram_tensor("gammaT", [P, CB * D], f32, kind="ExternalInput")
    b_d = nc.dram_tensor("betaT", [P, CB * D], f32, kind="ExternalInput")
    YT_d = nc.dram_tensor("YT", [C, padrows], f16, kind="ExternalOutput")

    cc_in = nc.dram_tensor("cc_in", [D, 2 * C + 1], f32)
    cc_out = nc.dram_tensor("cc_out", [D, 2 * C + 1], f32,
                            addr_space="Shared")

    # stats layout: partition p owns rows [p*64, (p+1)*64); chunk i = rows
    # {p*64 + i}; a LOAD_CH tile is 4 consecutive rows -> 2KB contiguous
    Xv = Xs_d.ap().rearrange("(p n) c -> p n c", p=P)   # [128, 64, 512]
    X2v = X2_d.ap().rearrange("(p n) c -> p n c", p=P)

    with tile.TileContext(nc) as tc:
        with (
            tc.tile_pool(name="const", bufs=1) as cpool,
            tc.tile_pool(name="xt", bufs=CB * D // 2) as xtpool,
            tc.tile_pool(name="small", bufs=1) as spool,
            tc.tile_pool(name="scr", bufs=2) as scrpool,
        ):
            # ---- small loads + constants ----
            oh = cpool.tile([P, CHUNKS, D], fp8)
            nc.sync.dma_start(oh[:], OH_d.ap().rearrange(
                "p (n d) -> p n d", d=D))
            rinvT = spool.tile([P, CB * D], f32, tag="rinvT")
            nc.sync.dma_start(rinvT[:], ri_d[:])
            gamT = spool.tile([P, CB * D], f32, tag="gamT")
            nc.sync.dma_start(gamT[:], g_d[:])
            betT = spool.tile([P, CB * D], f32, tag="betT")
            nc.sync.dma_start(betT[:], b_d[:])
            # identity [16,16] f32 for the PE transposes
            iota_r = cpool.tile([D, D], i32)
            nc.gpsimd.iota(iota_r[:], pattern=[[1, D]], base=0,
                           channel_multiplier=0)
            iota_c = cpool.tile([D, 1], i32)
            nc.gpsimd.iota(iota_c[:], pattern=[[0, 1]], base=0,
                           channel_multiplier=1)
            iota_cf = cpool.tile([D, 1], f32)
            nc.vector.tensor_copy(iota_cf[:], iota_c[:])
            ident = cpool.tile([D, D], f32)
            nc.vector.tensor_scalar(ident[:], iota_r[:], iota_cf[:], None,
                                    Alu.is_equal)
            epsb = spool.tile([P, 1], f32, tag="epsb")
            nc.vector.memset(epsb[:], EPS)

            # ---- stats inputs: Xs/X2 4-chunk tiles, interleaved so both
            # streams of a pair arrive together; the x pool is scoped so
            # its 64KB/partition is reused by the phase-2 y pool ----
            with tc.tile_pool(name="x", bufs=12) as xpool:
                xs, x2 = [], []
                for s in range(CHUNKS // LOAD_CH):
                    t1 = xpool.tile([P, LOAD_CH * C], fp8)
                    xs.append(t1)
                    nc.sync.dma_start(
                        t1[:].rearrange("p (n c) -> p n c", c=C),
                        Xv[:, LOAD_CH * s:LOAD_CH * (s + 1), :])
                    t2 = xpool.tile([P, LOAD_CH * C], fp8)
                    x2.append(t2)
                    # scalar-engine trigger: issues in parallel with sync's
                    nc.scalar.dma_start(
                        t2[:].rearrange("p (n c) -> p n c", c=C),
                        X2v[:, LOAD_CH * s:LOAD_CH * (s + 1), :])

                # X^T pair-tiles ride the same sync queues AFTER Xs/X2
                xtt = {}
                for cb in range(CB):
                    for jp in range(D // 2):
                        j0 = 2 * jp
                        w = int(alloc[j0] + alloc[j0 + 1])
                        t = xtpool.tile([P, w], f16)
                        xtt[(cb, jp)] = t
                        nc.sync.dma_start(
                            t[:], XT_d.ap()[cb * P:(cb + 1) * P,
                                            offs[j0]:offs[j0 + 2]])

                def pair(lst, k):  # [128, 2, C] fp8 for chunks 2k, 2k+1
                    t = lst[(2 * k) // LOAD_CH]
                    o = ((2 * k) % LOAD_CH) * C
                    return t[:, o:o + 2 * C].rearrange("p (t c) -> p t c",
                                                       c=C)

                # phase 1: per-core partial sums via fp8 DoubleRow
                with tc.tile_pool(name="ps1", bufs=1, space="PSUM") as ps1:
                    psum_s = ps1.tile([D, C], f32)
                    psum_q = ps1.tile([D, C], f32)
                    for k in range(PAIRS):
                        st, sp = (k == 0), (k == PAIRS - 1)
                        lhs = oh[:, 2 * k:2 * k + 2, :]
                        nc.tensor.matmul(
                            psum_s[:], lhs, pair(xs, k), start=st, stop=sp,
                            perf_mode=mybir.MatmulPerfMode.DoubleRow)
                        nc.tensor.matmul(
                            psum_q[:], lhs, pair(x2, k), start=st, stop=sp,
                            perf_mode=mybir.MatmulPerfMode.DoubleRow)

                    # stats out: PSUM -> SBUF copies, then two parallel
                    # half-DMAs to cc_in on different trigger engines
                    stats = spool.tile([D, 2 * C + 1], f32, tag="stats")
                    nc.vector.memset(stats[:, 2 * C:2 * C + 1], 0.0)
                    nc.vector.tensor_copy(stats[:, 0:C], psum_s[:])
                    nc.vector.tensor_copy(stats[:, C:2 * C], psum_q[:])
                    # gpsimd queues are empty in phase 1 -> no FIFO backlog
                    nc.gpsimd.dma_start(cc_in.ap()[:, 0:C], stats[:, 0:C])
                    nc.scalar.dma_start(cc_in.ap()[:, C:2 * C + 1],
                                        stats[:, C:2 * C + 1])

            # ---- all-reduce partial sums ----
            nc.gpsimd.collective_compute(
                "AllReduce", Alu.add,
                replica_groups=[list(range(NCORES))],
                ins=[cc_in[:]], outs=[cc_out[:]])

            # ---- read back + transpose per 128-column block: 8 small DMAs
            # on alternating engines, each feeding its own PE transpose as
            # soon as it lands (redT[p, b*16+j] = red[j, b*128+p]) ----
            redT = spool.tile([P, 2 * CB * D], f32, tag="redT")
            with tc.tile_pool(name="ps2", bufs=1, space="PSUM") as ps2:
                pt = ps2.tile([P, 2 * CB * D], f32)
                for b in range(2 * CB):
                    rb = scrpool.tile([D, P], f32, tag=f"red{b % 2}")
                    eng = nc.scalar if b % 2 == 0 else nc.gpsimd
                    eng.dma_start(rb[:], cc_out.ap()[:, b * P:(b + 1) * P])
                    nc.tensor.matmul(pt[:, b * D:(b + 1) * D],
                                     rb[:], ident[:],
                                     start=True, stop=True)
                nc.vector.tensor_copy(redT[:], pt[:])

            # ---- finalize in [128, 64]: A = inv*gamma, B = beta - mean*A
            F = CB * D
            meanT = spool.tile([P, F], f32, tag="meanT")
            nc.vector.tensor_mul(meanT[:], redT[:, 0:F], rinvT[:])
            varT = spool.tile([P, F], f32, tag="varT")
            nc.vector.tensor_mul(varT[:], redT[:, F:2 * F], rinvT[:])
            negm2 = scrpool.tile([P, F], f32, tag="scr")
            nc.vector.scalar_tensor_tensor(negm2[:], meanT[:], -1.0, meanT[:],
                                           Alu.mult, Alu.mult)
            nc.vector.tensor_add(varT[:], varT[:], negm2[:])
            sdT = scrpool.tile([P, F], f32, tag="scr")
            nc.scalar.activation(sdT[:], varT[:], Act.Sqrt, bias=epsb[:])
            invT = spool.tile([P, F], f32, tag="invT")
            nc.vector.reciprocal(invT[:], sdT[:])
            AT = spool.tile([P, F], f32, tag="AT")
            nc.vector.tensor_mul(AT[:], invT[:], gamT[:])
            BT = spool.tile([P, F], f32, tag="BT")
            nc.vector.scalar_tensor_tensor(BT[:], meanT[:], -1.0, AT[:],
                                           Alu.mult, Alu.mult)   # -mean*A
            nc.vector.tensor_add(BT[:], betT[:], BT[:])

            def a_col(cb, j):
                return AT[:, cb * D + j:cb * D + j + 1]

            def b_col(cb, j):
                return BT[:, cb * D + j:cb * D + j + 1]

            # ---- phase 2: y^T = x^T * A[d,c] + B[d,c] per rectangle ----
            # greedy DVE/ACT balance (ns/col measured on hw; DVE starts
            # loaded with the finalize chain)
            with tc.tile_pool(name="y", bufs=16) as ypool:
                dve_t, act_t = 2500.0, 0.0
                for cb in range(CB):
                    for jp in range(D // 2):
                        j0 = 2 * jp
                        w0 = int(alloc[j0])
                        w = int(alloc[j0] + alloc[j0 + 1])
                        xt2 = xtt[(cb, jp)]
                        yt = ypool.tile([P, w], f16)
                        for h, j in ((0, j0), (1, j0 + 1)):
                            sl = slice(0, w0) if h == 0 else slice(w0, w)
                            wj = int(alloc[j])
                            cd, ca = wj * 0.807, wj * 1.72
                            if dve_t + cd <= act_t + ca:
                                dve_t += cd
                                nc.vector.tensor_scalar(yt[:, sl], xt2[:, sl],
                                                        a_col(cb, j),
                                                        b_col(cb, j),
                                                        Alu.mult, Alu.add)
                            else:
                                act_t += ca
                                nc.scalar.activation(yt[:, sl], xt2[:, sl],
                                                     Act.Identity,
                                                     bias=b_col(cb, j),
                                                     scale=a_col(cb, j))
                        eng = nc.sync if jp % 2 == 0 else nc.gpsimd
                        eng.dma_start(
                            YT_d.ap()[cb * P:(cb + 1) * P,
                                      offs[j0]:offs[j0 + 2]],
                            yt[:])

    nc.compile()
    return nc


def _get_program(alloc):
    key = tuple(int(a) for a in alloc)
    if key not in _CACHE:
        _CACHE[key] = _build_program(alloc)
    return _CACHE[key]


def _plan(d):
    """Per-core, per-domain row assignment (SPMD-uniform allocation)."""
    cnt = np.bincount(d, minlength=D).astype(np.int64)
    # per-core allocation for domain j, rounded up to 32 rows, min 32
    alloc = np.maximum(32, ((cnt + NCORES - 1) // NCORES + 31) // 32 * 32)
    order = np.argsort(d, kind="stable")
    splits = np.cumsum(cnt)[:-1]
    by_dom = np.split(order, splits)          # global row ids per domain
    padrows = int(alloc.sum())
    perm = np.empty((NCORES, padrows), dtype=np.int64)
    valid = np.zeros((NCORES, padrows), dtype=bool)
    offs = np.concatenate([[0], np.cumsum(alloc)]).astype(int)
    for j in range(D):
        rows = by_dom[j]
        cuts = np.linspace(0, len(rows), NCORES + 1).astype(np.int64)
        for c in range(NCORES):
            part = rows[cuts[c]:cuts[c + 1]]
            n = len(part)
            o = offs[j]
            if n:
                perm[c, o:o + n] = part
                valid[c, o:o + n] = True
                perm[c, o + n:offs[j + 1]] = part[0]  # pad = repeat
            else:
                perm[c, o:offs[j + 1]] = 0            # inert; masked out
    return alloc, perm, valid, cnt


def _prepare(X, d, gamma, beta):
    """Build (nc, in_maps, plan) for the given full inputs."""
    import ml_dtypes

    X = np.ascontiguousarray(np.asarray(X), dtype=np.float32)
    d = np.ascontiguousarray(np.asarray(d), dtype=np.int32)
    gamma = np.ascontiguousarray(np.asarray(gamma), dtype=np.float32)
    beta = np.ascontiguousarray(np.asarray(beta), dtype=np.float32)

    alloc, perm, valid, cnt = _plan(d)
    nc = _get_program(alloc)

    X8 = X.astype(ml_dtypes.float8_e4m3)
    X28 = (X * X).astype(ml_dtypes.float8_e4m3)
    Xh = X.astype(np.float16)
    rinv = (1.0 / np.maximum(cnt, 1)).astype(np.float32)

    # transposed per-partition constants: t[p, cb*16+j] over channels
    # c = cb*128+p
    def tconst(M):  # M [D, C] -> [P, CB*D]
        out = np.empty((P, CB * D), dtype=np.float32)
        for cb in range(CB):
            out[:, cb * D:(cb + 1) * D] = M[:, cb * P:(cb + 1) * P].T
        return np.ascontiguousarray(out)

    rinvT = np.ascontiguousarray(
        np.tile(rinv[None, :], (P, CB)).astype(np.float32))
    gamT = tconst(gamma)
    betT = tconst(beta)

    in_maps = []
    for c in range(NCORES):
        ds = d[c * SHARD:(c + 1) * SHARD]
        dv = ds.reshape(P, CHUNKS)                      # row p*64+i
        ohc = (dv[:, :, None] == np.arange(D)[None, None, :])
        ohc = np.ascontiguousarray(
            ohc.reshape(P, CHUNKS * D).astype(ml_dtypes.float8_e4m3))
        xtc = np.ascontiguousarray(Xh[perm[c]].T)       # [C, padrows] f16
        in_maps.append({
            "Xs": X8[c * SHARD:(c + 1) * SHARD],
            "X2": X28[c * SHARD:(c + 1) * SHARD],
            "OH": ohc,
            "XT": xtc,
            "rinvT": rinvT,
            "gammaT": gamT,
            "betaT": betT,
        })
    return nc, in_maps, (perm, valid)


def _unpack(res, plan):
    perm, valid = plan
    Y = np.empty((N, C), dtype=np.float32)
    for c in range(NCORES):
        yt = np.asarray(res.results[c]["YT"]).astype(np.float32)  # [C, pad]
        m = valid[c]
        Y[perm[c][m]] = yt.T[m]
    return Y


def kernel(X, d, parameter_t, fm_mean, gamma, beta):
    from concourse.bass_utils import run_bass_kernel_spmd

    nc, in_maps, plan = _prepare(X, d, gamma, beta)
    res = run_bass_kernel_spmd(nc, in_maps, core_ids=list(range(NCORES)))
    return _unpack(res, plan)


# revision 28
# speedup vs baseline: 1.5488x; 1.0392x over previous
"""Per-domain batch normalization (BaseDomainBatchNorm) on 8 Trainium2 NeuronCores.

Math (reference):
    cnt[j]   = #{n : d[n] == j}            (clamped to >= 1)
    mean[j]  = sum_{d[n]==j} X[n] / cnt[j]
    var[j]   = sum_{d[n]==j} X[n]^2 / cnt[j] - mean[j]^2
    inv[j]   = rsqrt(var[j] + 1e-5)
    Y[n]     = (X[n] - mean[d[n]]) * inv[d[n]] * gamma[d[n]] + beta[d[n]]
             = X[n] * A[d[n]] + B[d[n]],  A = inv*gamma, B = beta - mean*A

Sharding: rows split 8192 per core; per-domain partial stats (sum/sumsq)
AllReduce'd across the 8 cores; each core normalizes its own rows.

v4 design
---------
Stats path (row-major, original order): host uploads X and X^2 in fp8 e4m3
plus the row one-hot of d; partial sums are 64 fp8 DoubleRow matmuls
(2 chunks = 2 K-tiles per pass).  Counts are exact on the host; 1/cnt
ships as a constant.  The AllReduce payload is [16, 2C+1] f32 (65600 B —
kept above 64 KiB so the runtime picks the faster RDH algorithm).

Normalize path (transposed, domain-sorted): host sorts rows by domain and
uploads X^T in fp16 (channels on partitions) with each domain's rows
padded to a fixed per-core allocation; every (channel-block, domain)
rectangle is a compile-time slice whose A/B are per-partition [128,1]
scalars, so the normalize is ONE fused mul-add per rectangle (DVE
tensor_scalar ~0.8ns/elem, shared 2:1 with the scalar engine's Identity
activation).  Y^T returns in fp16 and is unsorted/upcast on the host.

The finalize runs in the transposed [128 x 64] layout: the reduced stats
are PE-transposed against a [16,16] identity right after the AllReduce,
gamma/beta/1-over-cnt arrive pre-transposed from the host, and the
reciprocal runs on 128 partitions (8x shorter).

DMA discipline (one queue per DMA instruction, ~23 GB/s each): stats
tiles are 16+16 x 256 KB interleaved; X^T rides the same sync-engine
queues BEHIND them (FIFO) as 32 pair-tiles; Y writes are 32 pair-tiles.
The collective input/output DMAs trigger from the scalar engine so they
never queue behind bulk traffic.  Program is compiled per domain-
allocation tuple (derived from d) and cached.
"""

import numpy as np

N = 65536
C = 512
D = 16
NCORES = 8
SHARD = N // NCORES          # 8192 rows per core
P = 128                      # partitions
CHUNKS = SHARD // P          # 64 chunks of 128 rows
PAIRS = CHUNKS // 2          # 32 DoubleRow K-tile pairs
LOAD_CH = 4                  # chunks per stats DMA tile
CB = C // P                  # 4 channel blocks
EPS = 1e-5

_CACHE = {}


def _build_program(alloc):
    import concourse.bacc as bacc
    import concourse.tile as tile
    from concourse import mybir

    f32 = mybir.dt.float32
    f16 = mybir.dt.float16
    fp8 = mybir.dt.float8e4
    i32 = mybir.dt.int32
    Alu = mybir.AluOpType
    Act = mybir.ActivationFunctionType

    padrows = sum(alloc)
    offs = np.concatenate([[0], np.cumsum(alloc)]).astype(int)

    nc = bacc.Bacc("TRN2", target_bir_lowering=False, debug=False,
                   num_devices=NCORES)

    Xs_d = nc.dram_tensor("Xs", [SHARD, C], fp8, kind="ExternalInput")
    X2_d = nc.dram_tensor("X2", [SHARD, C], fp8, kind="ExternalInput")
    OH_d = nc.dram_tensor("OH", [P, CHUNKS * D], fp8, kind="ExternalInput")
    XT_d = nc.dram_tensor("XT", [C, padrows], f16, kind="ExternalInput")
    ri_d = nc.dram_tensor("rinvT", [P, CB * D], f32, kind="ExternalInput")
    g_d = nc.dram_tensor("gammaT", [P, CB * D], f32, kind="ExternalInput")
    b_d = nc.dram_tensor("betaT", [P, CB * D], f32, kind="ExternalInput")
    YT_d = nc.dram_tensor("YT", [C, padrows], f16, kind="ExternalOutput")

    cc_in = nc.dram_tensor("cc_in", [D, 2 * C + 1], f32)
    cc_out = nc.dram_tensor("cc_out", [D, 2 * C + 1], f32,
                            addr_space="Shared")

    # stats layout: partition p owns rows [p*64, (p+1)*64); chunk i = rows
    # {p*64 + i}; a LOAD_CH tile is 4 consecutive rows -> 2KB contiguous
    Xv = Xs_d.ap().rearrange("(p n) c -> p n c", p=P)   # [128, 64, 512]
    X2v = X2_d.ap().rearrange("(p n) c -> p n c", p=P)

    with tile.TileContext(nc) as tc:
        with (
            tc.tile_pool(name="const", bufs=1) as cpool,
            tc.tile_pool(name="xt", bufs=CB * D // 2) as xtpool,
            tc.tile_pool(name="small", bufs=1) as spool,
            tc.tile_pool(name="scr", bufs=2) as scrpool,
        ):
            # ---- small loads + constants ----
            oh = cpool.tile([P, CHUNKS, D], fp8)
            nc.sync.dma_start(oh[:], OH_d.ap().rearrange(
                "p (n d) -> p n d", d=D))
            rinvT = spool.tile([P, CB * D], f32, tag="rinvT")
            nc.sync.dma_start(rinvT[:], ri_d[:])
            gamT = spool.tile([P, CB * D], f32, tag="gamT")
            nc.sync.dma_start(gamT[:], g_d[:])
            betT = spool.tile([P, CB * D], f32, tag="betT")
            nc.sync.dma_start(betT[:], b_d[:])
            # identity [128,128] f32 for the one-shot PE transpose
            iota_r = cpool.tile([P, P], i32)
            nc.gpsimd.iota(iota_r[:], pattern=[[1, P]], base=0,
                           channel_multiplier=0)
            iota_c = cpool.tile([P, 1], i32)
            nc.gpsimd.iota(iota_c[:], pattern=[[0, 1]], base=0,
                           channel_multiplier=1)
            iota_cf = cpool.tile([P, 1], f32)
            nc.vector.tensor_copy(iota_cf[:], iota_c[:])
            ident = cpool.tile([P, P], f32)
            nc.vector.tensor_scalar(ident[:], iota_r[:], iota_cf[:], None,
                                    Alu.is_equal)
            epsb = spool.tile([P, 1], f32, tag="epsb")
            nc.vector.memset(epsb[:], EPS)

            # ---- stats inputs: Xs/X2 4-chunk tiles, interleaved so both
            # streams of a pair arrive together; the x pool is scoped so
            # its 64KB/partition is reused by the phase-2 y pool ----
            with tc.tile_pool(name="x", bufs=12) as xpool:
                xs, x2 = [], []
                for s in range(CHUNKS // LOAD_CH):
                    t1 = xpool.tile([P, LOAD_CH * C], fp8)
                    xs.append(t1)
                    nc.sync.dma_start(
                        t1[:].rearrange("p (n c) -> p n c", c=C),
                        Xv[:, LOAD_CH * s:LOAD_CH * (s + 1), :])
                    t2 = xpool.tile([P, LOAD_CH * C], fp8)
                    x2.append(t2)
                    # scalar-engine trigger: issues in parallel with sync's
                    nc.scalar.dma_start(
                        t2[:].rearrange("p (n c) -> p n c", c=C),
                        X2v[:, LOAD_CH * s:LOAD_CH * (s + 1), :])

                # X^T pair-tiles ride the same sync queues AFTER Xs/X2
                xtt = {}
                for cb in range(CB):
                    for jp in range(D // 2):
                        j0 = 2 * jp
                        w = int(alloc[j0] + alloc[j0 + 1])
                        t = xtpool.tile([P, w], f16)
                        xtt[(cb, jp)] = t
                        nc.sync.dma_start(
                            t[:], XT_d.ap()[cb * P:(cb + 1) * P,
                                            offs[j0]:offs[j0 + 2]])

                def pair(lst, k):  # [128, 2, C] fp8 for chunks 2k, 2k+1
                    t = lst[(2 * k) // LOAD_CH]
                    o = ((2 * k) % LOAD_CH) * C
                    return t[:, o:o + 2 * C].rearrange("p (t c) -> p t c",
                                                       c=C)

                # phase 1: per-core partial sums via fp8 DoubleRow
                with tc.tile_pool(name="ps1", bufs=1, space="PSUM") as ps1:
                    psum_s = ps1.tile([D, C], f32)
                    psum_q = ps1.tile([D, C], f32)
                    for k in range(PAIRS):
                        st, sp = (k == 0), (k == PAIRS - 1)
                        lhs = oh[:, 2 * k:2 * k + 2, :]
                        nc.tensor.matmul(
                            psum_s[:], lhs, pair(xs, k), start=st, stop=sp,
                            perf_mode=mybir.MatmulPerfMode.DoubleRow)
                        nc.tensor.matmul(
                            psum_q[:], lhs, pair(x2, k), start=st, stop=sp,
                            perf_mode=mybir.MatmulPerfMode.DoubleRow)

                    # stats out: PSUM -> SBUF copies, then two parallel
                    # half-DMAs to cc_in on different trigger engines
                    stats = spool.tile([D, 2 * C + 1], f32, tag="stats")
                    nc.vector.memset(stats[:, 2 * C:2 * C + 1], 0.0)
                    nc.vector.tensor_copy(stats[:, 0:C], psum_s[:])
                    nc.vector.tensor_copy(stats[:, C:2 * C], psum_q[:])
                    # gpsimd queues are empty in phase 1 -> no FIFO backlog
                    nc.gpsimd.dma_start(cc_in.ap()[:, 0:C], stats[:, 0:C])
                    nc.scalar.dma_start(cc_in.ap()[:, C:2 * C + 1],
                                        stats[:, C:2 * C + 1])

            # ---- all-reduce partial sums ----
            nc.gpsimd.collective_compute(
                "AllReduce", Alu.add,
                replica_groups=[list(range(NCORES))],
                ins=[cc_in[:]], outs=[cc_out[:]])

            # ---- read back + transpose: 8 parallel DMAs land the reduced
            # stats STACKED on partitions (partition 16b+j = block b of
            # domain j), then ONE K=128 matmul against the identity
            # transposes everything: redT[p, 16b+j] = red[j, 128b+p] ----
            red_st = spool.tile([P, P], f32, tag="red_st")
            for b in range(2 * CB):
                eng = nc.scalar if b % 2 == 0 else nc.gpsimd
                eng.dma_start(red_st[b * D:(b + 1) * D, :],
                              cc_out.ap()[:, b * P:(b + 1) * P])
            redT = spool.tile([P, 2 * CB * D], f32, tag="redT")
            with tc.tile_pool(name="ps2", bufs=1, space="PSUM") as ps2:
                pt = ps2.tile([P, 2 * CB * D], f32)
                nc.tensor.matmul(pt[:], red_st[:], ident[:],
                                 start=True, stop=True)
                nc.vector.tensor_copy(redT[:], pt[:])

            # ---- finalize in [128, 64]: A = inv*gamma, B = beta - mean*A
            F = CB * D
            meanT = spool.tile([P, F], f32, tag="meanT")
            nc.vector.tensor_mul(meanT[:], redT[:, 0:F], rinvT[:])
            varT = spool.tile([P, F], f32, tag="varT")
            nc.vector.tensor_mul(varT[:], redT[:, F:2 * F], rinvT[:])
            negm2 = scrpool.tile([P, F], f32, tag="scr")
            nc.vector.scalar_tensor_tensor(negm2[:], meanT[:], -1.0, meanT[:],
                                           Alu.mult, Alu.mult)
            nc.vector.tensor_add(varT[:], varT[:], negm2[:])
            sdT = scrpool.tile([P, F], f32, tag="scr")
            nc.scalar.activation(sdT[:], varT[:], Act.Sqrt, bias=epsb[:])
            invT = spool.tile([P, F], f32, tag="invT")
            nc.vector.reciprocal(invT[:], sdT[:])
            AT = spool.tile([P, F], f32, tag="AT")
            nc.vector.tensor_mul(AT[:], invT[:], gamT[:])
            BT = spool.tile([P, F], f32, tag="BT")
            nc.vector.scalar_tensor_tensor(BT[:], meanT[:], -1.0, AT[:],
                                           Alu.mult, Alu.mult)   # -mean*A
            nc.vector.tensor_add(BT[:], betT[:], BT[:])

            def a_col(cb, j):
                return AT[:, cb * D + j:cb * D + j + 1]

            def b_col(cb, j):
                return BT[:, cb * D + j:cb * D + j + 1]

            # ---- phase 2: y^T = x^T * A[d,c] + B[d,c] per rectangle ----
            # greedy DVE/ACT balance (ns/col measured on hw; DVE starts
            # loaded with the finalize chain)
            with tc.tile_pool(name="y", bufs=24) as ypool:
                dve_t, act_t = 2500.0, 0.0
                for cb in range(CB):
                    for jp in range(D // 2):
                        j0 = 2 * jp
                        w0 = int(alloc[j0])
                        w = int(alloc[j0] + alloc[j0 + 1])
                        xt2 = xtt[(cb, jp)]
                        yt = ypool.tile([P, w], f16)
                        for h, j in ((0, j0), (1, j0 + 1)):
                            sl = slice(0, w0) if h == 0 else slice(w0, w)
                            wj = int(alloc[j])
                            cd, ca = wj * 0.807, wj * 1.72
                            if dve_t + cd <= act_t + ca:
                                dve_t += cd
                                nc.vector.tensor_scalar(yt[:, sl], xt2[:, sl],
                                                        a_col(cb, j),
                                                        b_col(cb, j),
                                                        Alu.mult, Alu.add)
                            else:
                                act_t += ca
                                nc.scalar.activation(yt[:, sl], xt2[:, sl],
                                                     Act.Identity,
                                                     bias=b_col(cb, j),
                                                     scale=a_col(cb, j))
                        eng = nc.sync if jp % 2 == 0 else nc.gpsimd
                        eng.dma_start(
                            YT_d.ap()[cb * P:(cb + 1) * P,
                                      offs[j0]:offs[j0 + 2]],
                            yt[:])

    nc.compile()
    return nc


def _get_program(alloc):
    key = tuple(int(a) for a in alloc)
    if key not in _CACHE:
        _CACHE[key] = _build_program(alloc)
    return _CACHE[key]


def _plan(d):
    """Per-core, per-domain row assignment (SPMD-uniform allocation)."""
    cnt = np.bincount(d, minlength=D).astype(np.int64)
    # per-core allocation for domain j, rounded up to 32 rows, min 32
    alloc = np.maximum(32, ((cnt + NCORES - 1) // NCORES + 31) // 32 * 32)
    order = np.argsort(d, kind="stable")
    splits = np.cumsum(cnt)[:-1]
    by_dom = np.split(order, splits)          # global row ids per domain
    padrows = int(alloc.sum())
    perm = np.empty((NCORES, padrows), dtype=np.int64)
    valid = np.zeros((NCORES, padrows), dtype=bool)
    offs = np.concatenate([[0], np.cumsum(alloc)]).astype(int)
    for j in range(D):
        rows = by_dom[j]
        cuts = np.linspace(0, len(rows), NCORES + 1).astype(np.int64)
        for c in range(NCORES):
            part = rows[cuts[c]:cuts[c + 1]]
            n = len(part)
            o = offs[j]
            if n:
                perm[c, o:o + n] = part
                valid[c, o:o + n] = True
                perm[c, o + n:offs[j + 1]] = part[0]  # pad = repeat
            else:
                perm[c, o:offs[j + 1]] = 0            # inert; masked out
    return alloc, perm, valid, cnt


def _prepare(X, d, gamma, beta):
    """Build (nc, in_maps, plan) for the given full inputs."""
    import ml_dtypes

    X = np.ascontiguousarray(np.asarray(X), dtype=np.float32)
    d = np.ascontiguousarray(np.asarray(d), dtype=np.int32)
    gamma = np.ascontiguousarray(np.asarray(gamma), dtype=np.float32)
    beta = np.ascontiguousarray(np.asarray(beta), dtype=np.float32)

    alloc, perm, valid, cnt = _plan(d)
    nc = _get_program(alloc)

    X8 = X.astype(ml_dtypes.float8_e4m3)
    X28 = (X * X).astype(ml_dtypes.float8_e4m3)
    Xh = X.astype(np.float16)
    rinv = (1.0 / np.maximum(cnt, 1)).astype(np.float32)

    # transposed per-partition constants: t[p, cb*16+j] over channels
    # c = cb*128+p
    def tconst(M):  # M [D, C] -> [P, CB*D]
        out = np.empty((P, CB * D), dtype=np.float32)
        for cb in range(CB):
            out[:, cb * D:(cb + 1) * D] = M[:, cb * P:(cb + 1) * P].T
        return np.ascontiguousarray(out)

    rinvT = np.ascontiguousarray(
        np.tile(rinv[None, :], (P, CB)).astype(np.float32))
    gamT = tconst(gamma)
    betT = tconst(beta)

    in_maps = []
    for c in range(NCORES):
        ds = d[c * SHARD:(c + 1) * SHARD]
        dv = ds.reshape(P, CHUNKS)                      # row p*64+i
        ohc = (dv[:, :, None] == np.arange(D)[None, None, :])
        ohc = np.ascontiguousarray(
            ohc.reshape(P, CHUNKS * D).astype(ml_dtypes.float8_e4m3))
        xtc = np.ascontiguousarray(Xh[perm[c]].T)       # [C, padrows] f16
        in_maps.append({
            "Xs": X8[c * SHARD:(c + 1) * SHARD],
            "X2": X28[c * SHARD:(c + 1) * SHARD],
            "OH": ohc,
            "XT": xtc,
            "rinvT": rinvT,
            "gammaT": gamT,
            "betaT": betT,
        })
    return nc, in_maps, (perm, valid)


def _unpack(res, plan):
    perm, valid = plan
    Y = np.empty((N, C), dtype=np.float32)
    for c in range(NCORES):
        yt = np.asarray(res.results[c]["YT"]).astype(np.float32)  # [C, pad]
        m = valid[c]
        Y[perm[c][m]] = yt.T[m]
    return Y


def kernel(X, d, parameter_t, fm_mean, gamma, beta):
    from concourse.bass_utils import run_bass_kernel_spmd

    nc, in_maps, plan = _prepare(X, d, gamma, beta)
    res = run_bass_kernel_spmd(nc, in_maps, core_ids=list(range(NCORES)))
    return _unpack(res, plan)
